# revision 36
# baseline (speedup 1.0000x reference)
"""Stochastic-LIF neuron kernel for Trainium2 (8 NeuronCores).

Reference recurrence per element (b, n), over T=128 time steps:
    u_t = 0.5 * u_{t-1} + x_t
    o_t = (u_t > 1)
    u_t = u_t * (1 - o_t)        # hard reset to 0 on spike

Strategy:
  - Shard batch dim B=32 across 8 cores (4 per core). Per core the
    elements form a [128 partitions, 256 free] tile (4 b x 8192 n).
  - State kept as v (pre-reset potential). One fused custom DVE op per
    time step: v' = 0.5 * select(v <= 1, v, 0) + x_t   (~1 elem/cycle).
  - Spike output o = sign(v' - 1) on the ACT engine, saturating
    float->uint8 conversion maps {-1,0,1} -> {0,0,1} = (v' > 1).
  - x streamed in / o streamed out in CHUNK_T-step chunks, u8 output
    (4x less DMA); host converts to float32.
"""

import os

import numpy as np

B, T, N = 32, 128, 8192
NCORES = 8
BPC = B // NCORES          # batches per core
P = 128                    # SBUF partitions
F = BPC * N // P           # free dim per step = 256
PPB = P // BPC             # partition rows per batch = 32

_cache = {}
VARIANT = "full"           # production variant: "full" or "pack"


def _register_custom_op():
    import concourse.dve_ops as dve_ops

    if "LIF_STEP_ANT" in dve_ops._SUB_OPCODE_FOR_NAME:
        return next(op for op in dve_ops.OPS if op.name == "LIF_STEP_ANT")

    from concourse.dve_spec import C0, C1, Spec, Src0, Src1, Zero, select

    def _ref(in0, in1, s0, s1, imm2):
        u = np.where(in0 <= s1, in0, 0.0).astype(np.float32)
        return (u * s0 + in1).astype(np.float32)

    op = dve_ops.DveOp(
        "LIF_STEP_ANT",
        Spec(body=select(Src0 <= C1, Src0, Zero) * C0 + Src1, reference=_ref),
        subdim=False,
        uops_sha={"v3": "73713d2c766d7eeb", "v4": "f73a18201e32e28c"},
    )
    dve_ops.OPS.append(op)
    dve_ops.CUSTOM_DVE_SPECS[op.name] = op.spec
    dve_ops._SUB_OPCODE_FOR_NAME[op.name] = (
        dve_ops._CUSTOM_DVE_ROW_BASE + len(dve_ops.OPS) - 1
    )
    return op


def _build_nc(repeat=1, variant="full", mid_ct=16):
    import concourse.bacc as bacc
    import concourse.mybir as mybir
    from concourse.tile import TileContext

    lif_op = _register_custom_op()

    nc = bacc.Bacc()
    f32 = mybir.dt.float32
    u8 = mybir.dt.uint8

    # both tensors in [partition, t*F] device layout (per-partition time
    # history contiguous); host pre/post-transposes (free for HW time)
    pack = variant == "pack"
    x_d = nc.dram_tensor("x", [P, T * F], f32, kind="ExternalInput")
    o_cols = T * F // 8 if pack else T * F
    o_d = nc.dram_tensor("o", [P, o_cols], u8, kind="ExternalOutput")

    x_v = x_d[:].rearrange("p (t f) -> p t f", f=F)
    o_v = o_d[:].rearrange("p (t f) -> p t f", f=(F // 8 if pack else F))

    # variable chunk schedule: small chunks at start (fast pipeline fill)
    # and end (short drain), large in the middle
    chunk_ts = [4, 8] + [mid_ct] * ((T - 16) // mid_ct) + [4]
    assert sum(chunk_ts) == T
    with TileContext(nc) as tc:
        with (
            tc.tile_pool(name="xin", bufs=5) as xpool,
            tc.tile_pool(name="oout", bufs=3) as opool,
            tc.tile_pool(name="state", bufs=3) as vpool,
            tc.tile_pool(name="consts", bufs=1) as cpool,
            tc.tile_pool(name="packs", bufs=3) as ppool,
        ):
            bias_m1 = cpool.tile([P, 1], f32, tag="bias")
            nc.vector.memset(bias_m1[:], -1.0)
            z0 = cpool.tile([P, F], f32, tag="z0")
            nc.vector.memset(z0[:], 0.0)
            for _rep in range(repeat):
                v_prev = z0[:]
                t0 = 0
                for ct in chunk_ts:
                    xt = xpool.tile([P, ct * F], f32, tag="x")
                    xt3 = xt[:].rearrange("p (t f) -> p t f", f=F)
                    ot = opool.tile([P, ct * F], u8, tag="o")
                    # v history: ct states side by side
                    vh = vpool.tile([P, ct * F], f32, tag="v")
                    vh3 = vh[:].rearrange("p (t f) -> p t f", f=F)
                    nc.sync.dma_start(
                        out=xt[:],
                        in_=x_v[:, t0 : t0 + ct],
                    )
                    if variant in ("full", "noout", "pack"):
                        for j in range(ct):
                            nc.vector._custom_dve(
                                lif_op,
                                out=vh3[:, j],
                                in0=v_prev,
                                in1=xt3[:, j],
                                s0=0.5,
                                s1=1.0,
                            )
                            v_prev = vh3[:, j]
                        spike_src = vh
                    else:  # "nolif": ablation, spike straight from x
                        spike_src = xt
                    # one wide spike op per chunk:
                    # o = sign(v-1) in {-1,0,1}; f32->u8 saturates -> (v>1)
                    nc.scalar.activation(
                        ot[:],
                        spike_src[:],
                        mybir.ActivationFunctionType.Sign,
                        bias=bias_m1[:],
                        scale=1.0,
                    )
                    # out-DMA on the ACT queue: no head-of-line blocking of
                    # the SP queue's in-DMA prefetch for later chunks
                    if variant in ("full", "nolif"):
                        nc.scalar.dma_start(
                            out=o_v[:, t0 : t0 + ct],
                            in_=ot[:],
                        )
                    elif pack:
                        # bit-pack 8 spikes/byte on the idle GPSIMD engine:
                        # 3 pairwise shift-add levels, little-endian bits
                        w = ct * F
                        p1 = ppool.tile([P, w // 2], u8, tag="p1")
                        nc.gpsimd.scalar_tensor_tensor(
                            out=p1[:], in0=ot[:, 1::2], scalar=2.0,
                            in1=ot[:, 0::2],
                            op0=mybir.AluOpType.mult, op1=mybir.AluOpType.add,
                        )
                        p2 = ppool.tile([P, w // 4], u8, tag="p2")
                        nc.gpsimd.scalar_tensor_tensor(
                            out=p2[:], in0=p1[:, 1::2], scalar=4.0,
                            in1=p1[:, 0::2],
                            op0=mybir.AluOpType.mult, op1=mybir.AluOpType.add,
                        )
                        p3 = ppool.tile([P, w // 8], u8, tag="p3")
                        nc.gpsimd.scalar_tensor_tensor(
                            out=p3[:], in0=p2[:, 1::2], scalar=16.0,
                            in1=p2[:, 0::2],
                            op0=mybir.AluOpType.mult, op1=mybir.AluOpType.add,
                        )
                        nc.scalar.dma_start(
                            out=o_v[:, t0 : t0 + ct],
                            in_=p3[:],
                        )
                    t0 += ct
                if repeat > 1:
                    # decouple reps: reset state through a fresh zero tile
                    v_prev = z0[:]
    nc.compile()
    return nc


def _get_nc():
    if "nc" not in _cache:
        _cache["nc"] = _build_nc(variant=VARIANT)
    return _cache["nc"]


def kernel(x):
    from concourse.bass_utils import run_bass_kernel_spmd

    nc = _get_nc()
    x = np.asarray(x, dtype=np.float32)
    # host -> device layout: [b, t, (p f)] -> per-core [(b p), (t f)]
    xs = x.reshape(NCORES, BPC, T, PPB, F).transpose(0, 1, 3, 2, 4)
    xs = np.ascontiguousarray(xs).reshape(NCORES, P, T * F)
    in_maps = [{"x": xs[i]} for i in range(NCORES)]
    res = None
    for attempt in range(3):
        try:
            res = run_bass_kernel_spmd(
                nc,
                in_maps,
                core_ids=list(range(NCORES)),
                trace=bool(int(os.environ.get("LIF_TRACE", "0"))),
            )
            break
        except Exception:
            if attempt == 2:
                raise
    if res.exec_time_ns is not None:
        print(f"HW exec time: {res.exec_time_ns} ns")
        _cache["exec_time_ns"] = res.exec_time_ns
        _cache["trace"] = res.instructions_and_trace
    # device layout per core: [(b p), (t f)] -> host [b, t, (p f)]
    o = np.stack([res.results[i]["o"] for i in range(NCORES)])
    if VARIANT == "pack":
        o = np.unpackbits(o, axis=-1, bitorder="little")
    o = o.reshape(NCORES, BPC, PPB, T, F).transpose(0, 1, 3, 2, 4)
    return np.ascontiguousarray(o).reshape(B, T, N).astype(np.float32)
